# revision 18
# baseline (speedup 1.0000x reference)
"""Multi-head attention with scalar ALiBi slope, sharded over 8 NeuronCores.

Sharding: data-parallel over batch (B=2) x tensor-parallel over heads
(16 heads -> 4 per core).  Each core computes, for its batch element and
its 4 heads: Q/K/V projections, causal ALiBi attention over the full
T x T score matrix, and a partial out-projection over its 256 features.
Host sums the 4 partials per batch element and adds the output bias.

Hardcoded problem shape: B=2, T=2048, D=1024, H=16, hd=64, m=0.0625.
"""

import sys

sys.path.insert(0, "/opt/trn_rl_repo")

import numpy as np

import concourse.bass as bass
import concourse.bacc as bacc
import concourse.tile as tile
import concourse.mybir as mybir
from concourse.bass_utils import run_bass_kernel_spmd
from concourse.masks import make_identity

F32 = mybir.dt.float32
F32R = mybir.dt.float32r

B, T, D = 2, 2048, 1024
H_PER_CORE = 4
FPC = H_PER_CORE * 64          # features per core = 256
M_SLOPE = 0.0625
NT = T // 128                  # 16 row tiles
ND = D // 128                  # 8 contraction tiles

# When True, attn@V matmuls always span full 512-wide chunks (reading a
# zero-padded region of A below the causal diagonal), so PSUM accumulation
# start/stop flags are uniform per bank.  When False, sub-chunk matmuls skip
# the padded region (less PE work, non-uniform stop flags).
PAD_SAFE = True


def build_nc(trace_label=""):
    nc = bacc.Bacc(
        "TRN2",
        target_bir_lowering=False,
        debug=False,
        enable_asserts=False,
        num_devices=8,
    )
    xT = nc.dram_tensor("xT", [D, T], F32R, kind="ExternalInput").ap()
    wq = nc.dram_tensor("wq", [D, FPC], F32R, kind="ExternalInput").ap()
    wk = nc.dram_tensor("wk", [D, FPC], F32R, kind="ExternalInput").ap()
    wv = nc.dram_tensor("wv", [D, FPC], F32R, kind="ExternalInput").ap()
    bqv = nc.dram_tensor("bqv", [128, 2], F32, kind="ExternalInput").ap()
    bkv = nc.dram_tensor("bkv", [128, 2], F32, kind="ExternalInput").ap()
    bvv = nc.dram_tensor("bvv", [128, 2], F32, kind="ExternalInput").ap()
    wo = nc.dram_tensor("wo", [FPC, D], F32R, kind="ExternalInput").ap()
    ebias = nc.dram_tensor("ebias", [128, 2 * NT], F32, kind="ExternalInput").ap()
    umask = nc.dram_tensor("umask", [128, 128], F32R, kind="ExternalInput").ap()
    out = nc.dram_tensor("out", [T, D], F32, kind="ExternalOutput").ap()

    with tile.TileContext(nc) as tc:
        with (
            tc.tile_pool(name="const", bufs=1) as cst,
            tc.tile_pool(name="qk_sb", bufs=1) as qks,
            tc.tile_pool(name="v_sb", bufs=1) as vsb,
            tc.tile_pool(name="late", bufs=1) as late,
        ):
            umask_sb = cst.tile([128, 128], F32R, tag="umask")
            nc.sync.dma_start(out=umask_sb, in_=umask)
            ebias_sb = cst.tile([128, 2 * NT], F32, tag="ebias")
            nc.sync.dma_start(out=ebias_sb, in_=ebias)
            ident_f32 = cst.tile([128, 128], F32, tag="ident_f32")
            make_identity(nc, ident_f32)
            ident = cst.tile([128, 128], F32R, tag="ident")
            nc.vector.tensor_copy(ident, ident_f32)
            ones_f32 = cst.tile([128, 1], F32, tag="ones_f32")
            nc.vector.memset(ones_f32, 1.0)
            act_warm = cst.tile([128, 1], F32, tag="act_warm")
            nc.scalar.activation(
                act_warm, ones_f32, mybir.ActivationFunctionType.Exp
            )
            zeros_f32 = cst.tile([128, 384], F32, tag="zeros_f32")
            nc.vector.memset(zeros_f32, 0.0)
            bq_sb = cst.tile([128, 2], F32, tag="bq")
            nc.sync.dma_start(out=bq_sb, in_=bqv)
            bk_sb = cst.tile([128, 2], F32, tag="bk")
            nc.sync.dma_start(out=bk_sb, in_=bkv)
            bv_sb = cst.tile([128, 2], F32, tag="bv")
            nc.sync.dma_start(out=bv_sb, in_=bvv)

            # persistent QT/KT (f, T) and V natural (t, f-with-ones)
            qt = [qks.tile([128, T], F32R, tag=f"qt{ft}", name=f"qt{ft}") for ft in range(2)]
            kt = [qks.tile([128, T], F32R, tag=f"kt{ft}", name=f"kt{ft}") for ft in range(2)]
            v_t = [vsb.tile([128, 65 * H_PER_CORE], F32R, tag=f"v{t}", name=f"v{t}") for t in range(NT)]
            outT = [late.tile([128, T], F32R, tag=f"ot{ft}", name=f"ot{ft}") for ft in range(2)]
            wo_sb = [late.tile([128, D], F32R, tag=f"wo{ft}", name=f"wosb{ft}") for ft in range(2)]

            # ones columns of V (column 64 of each head's 65-wide slot)
            for t in range(NT):
                s = v_t[t][:, 64:65]
                ones_cols = bass.AP(
                    tensor=s.tensor, offset=s.offset, ap=[s.ap[0], [65, H_PER_CORE]]
                )
                o1 = ones_f32[:, 0:1]
                ones_src = bass.AP(
                    tensor=o1.tensor, offset=o1.offset, ap=[o1.ap[0], [0, H_PER_CORE]]
                )
                nc.vector.tensor_copy(ones_cols, ones_src)

            # ---------------- Phase 1: projections ----------------
            with (
                tc.tile_pool(name="xw", bufs=1) as xw,
                tc.tile_pool(name="ps1", bufs=1, space="PSUM") as ps1,
            ):
                x_sb = [xw.tile([128, T], F32R, tag=f"x{d}", name=f"x{d}") for d in range(ND)]
                w_sb = {}
                for nm in ("q", "k", "v"):
                    w_sb[nm] = [xw.tile([128, FPC], F32R, tag=f"w{nm}{d}", name=f"w{nm}{d}") for d in range(ND)]
                wdram = {"q": wq, "k": wk, "v": wv}
                for d in range(ND):
                    nc.sync.dma_start(
                        out=w_sb["q"][d], in_=wdram["q"][128 * d : 128 * (d + 1), :]
                    )
                for d in range(ND):
                    eng = nc.sync if d % 2 == 0 else nc.gpsimd
                    eng.dma_start(out=x_sb[d], in_=xT[128 * d : 128 * (d + 1), :])
                for nm in ("k", "v"):
                    for d in range(ND):
                        eng = nc.gpsimd if d % 2 == 0 else nc.sync
                        eng.dma_start(
                            out=w_sb[nm][d], in_=wdram[nm][128 * d : 128 * (d + 1), :]
                        )
                for ft in range(2):
                    nc.gpsimd.dma_start(
                        out=wo_sb[ft], in_=wo[128 * ft : 128 * (ft + 1), :]
                    )
                vt = [xw.tile([128, T], F32R, tag=f"vt{ft}", name=f"vt{ft}") for ft in range(2)]

                dsts = {"q": qt, "k": kt, "v": vt}
                bias = {"q": bq_sb, "k": bk_sb, "v": bv_sb}
                grp = 0
                for nm in ("q", "k", "v"):
                    for ft in range(2):
                        for half in range(2):
                            pj = [
                                ps1.tile([128, 512], F32, tag=f"pj{(2 * grp + i) % 3}", bufs=2, name=f"pj{grp}_{i}")
                                for i in range(2)
                            ]
                            grp += 1
                            for d in range(ND):
                                for i in range(2):
                                    c0 = 1024 * half + 512 * i
                                    nc.tensor.matmul(
                                        pj[i],
                                        (w_sb[nm][d][:, 128 * ft : 128 * (ft + 1)]),
                                        (x_sb[d][:, c0 : c0 + 512]),
                                        start=(d == 0),
                                        stop=(d == ND - 1),
                                    )
                            for i in range(2):
                                c0 = 1024 * half + 512 * i
                                nc.vector.tensor_scalar_add(
                                    dsts[nm][ft][:, c0 : c0 + 512],
                                    pj[i],
                                    bias[nm][:, ft : ft + 1],
                                )

                # V: transpose VT (f,T) -> V natural (t, f), head h in cols 65h..65h+63
                for ft in range(2):
                    for t in range(NT):
                        tp = ps1.tile([128, 128], F32R, tag="tp", bufs=2)
                        nc.tensor.transpose(
                            tp, vt[ft][:, 128 * t : 128 * (t + 1)], ident
                        )
                        # scatter 2 heads (64 cols each) into their 65-wide slots
                        d0 = v_t[t][:, 65 * 2 * ft : 65 * 2 * ft + 64]
                        dst = bass.AP(
                            tensor=d0.tensor,
                            offset=d0.offset,
                            ap=[d0.ap[0], [65, 2], [1, 64]],
                        )
                        src = tp.rearrange("p (h c) -> p h c", h=2)
                        nc.vector.tensor_copy(dst, src)

            # ---------------- Phase 2: attention ----------------
            with (
                tc.tile_pool(name="att", bufs=1) as att,
                tc.tile_pool(name="dr", bufs=2, space="DRAM") as drp,
                tc.tile_pool(name="ps2", bufs=1, space="PSUM") as ps2,
            ):
                recip = [
                    att.tile([1, T], F32, tag="recip", bufs=2, name=f"recip{h}")
                    for h in range(H_PER_CORE)
                ]
                for h in (2, 3, 0, 1):
                    ft, off = h // 2, 64 * (h % 2)
                    q_h = qt[ft][off : off + 64, :]
                    k_h = kt[ft][off : off + 64, :]
                    o_ps = ps2.tile([65, T], F32, tag="o", bufs=1)
                    for j in range(NT):
                        tk0 = 128 * j
                        c0 = j // 4          # first 512-chunk index
                        a0 = 512 * c0        # column base of a_tile
                        a_t = att.tile([128, T], F32R, tag="a", bufs=5)
                        # pieces split at the 1024 boundary (exp-bias offset blocks)
                        pieces = []
                        if tk0 < 1024:
                            pieces.append((tk0, 1024, 0))
                            pieces.append((1024, 2048, 1))
                        else:
                            pieces.append((tk0, 2048, 1))
                        for (t0, t1, bq_) in pieces:
                            w = t1 - t0
                            st = ps2.tile([128, 1024], F32, tag="st", bufs=2)
                            for cs in range(t0, t1, 512):
                                n = min(512, t1 - cs)
                                nc.tensor.matmul(
                                    st[:, cs - t0 : cs - t0 + n],
                                    (k_h[:, tk0 : tk0 + 128]),
                                    (q_h[:, cs : cs + n]),
                                    start=True,
                                    stop=True,
                                )
                            nc.scalar.activation(
                                a_t[:, t0 - a0 : t1 - a0],
                                st[:, 0:w],
                                mybir.ActivationFunctionType.Exp,
                                bias=ebias_sb[:, 2 * j + bq_ : 2 * j + bq_ + 1],
                            )
                        # zero the below-diagonal pad, mask the diagonal block
                        if PAD_SAFE and tk0 > a0:
                            nc.vector.tensor_copy(
                                a_t[:, 0 : tk0 - a0], zeros_f32[:, 0 : tk0 - a0]
                            )
                        nc.vector.tensor_mul(
                            a_t[:, tk0 - a0 : tk0 - a0 + 128],
                            a_t[:, tk0 - a0 : tk0 - a0 + 128],
                            umask_sb,
                        )
                        # attn @ [V | 1]
                        for c in range(c0, 4):
                            if PAD_SAFE:
                                cs = 512 * c
                            else:
                                cs = max(512 * c, tk0)
                            n = 512 * (c + 1) - cs
                            nc.tensor.matmul(
                                o_ps[:, cs : cs + n],
                                (v_t[j][:, 65 * h : 65 * h + 65]),
                                (a_t[:, cs - a0 : cs - a0 + n]),
                                start=(j == 0),
                                stop=(j == 4 * c + 3),
                            )
                    # stage PSUM accumulator to SBUF quickly so the next
                    # head's attn@V can claim the banks; normalize pipelined
                    # in two column halves to shorten the exposed chain
                    stage = att.tile([65, T], F32, tag="stage", bufs=2)
                    rb = att.tile([64, T], F32, tag="rb", bufs=2)
                    rdram = drp.tile([1, T], F32, tag="rd", bufs=2)
                    HT = T // 2
                    rr0 = att.tile([1, T], F32, tag="rr0", bufs=2)
                    for hf in range(2):
                        cs = slice(HT * hf, HT * (hf + 1))
                        nc.vector.tensor_copy(stage[:, cs], o_ps[:, cs])
                        # custom DVE op misreads at base partition 64 on HW:
                        # hop through a partition-0 row first
                        nc.vector.tensor_copy(rr0[:, cs], stage[64:65, cs])
                        nc.vector.reciprocal_approx_fast(
                            recip[h][:, cs], rr0[:, cs]
                        )
                        nc.sync.dma_start(out=rdram[:, cs], in_=recip[h][:, cs])
                        rbc = bass.AP(
                            tensor=rdram.tensor,
                            offset=rdram.offset + HT * hf,
                            ap=[[0, 64], [1, HT]],
                        )
                        nc.gpsimd.dma_start(out=rb[:, cs], in_=rbc)
                        nc.vector.tensor_mul(
                            outT[ft][off : off + 64, cs], stage[0:64, cs], rb[:, cs]
                        )

                # ---------------- Phase 3: out projection ----------------
                # po shares the "st" psum slots so no pool barrier separates
                # the attention tail from the first out-projection groups.
                for t in range(NT):
                    po = ps2.tile([128, D], F32, tag="st", bufs=2, name=f"po{t}")
                    for ft in (1, 0):
                        for c in range(2):
                            nc.tensor.matmul(
                                po[:, 512 * c : 512 * (c + 1)],
                                (outT[ft][:, 128 * t : 128 * (t + 1)]),
                                (wo_sb[ft][:, 512 * c : 512 * (c + 1)]),
                                start=(ft == 1),
                                stop=(ft == 0),
                            )
                    o_sb = att.tile([128, D], F32, tag="osb", bufs=3, name=f"osb{t}")
                    nc.vector.tensor_copy(o_sb, po)
                    eng = nc.sync if t % 2 == 0 else nc.gpsimd
                    eng.dma_start(out=out[128 * t : 128 * (t + 1), :], in_=o_sb)

    nc.finalize()
    return nc


_CACHE = {}


def _get_nc():
    if "nc" not in _CACHE:
        _CACHE["nc"] = build_nc()
    return _CACHE["nc"]


def _host_tables():
    i = np.arange(128, dtype=np.float32)[:, None]
    cols = np.arange(2 * NT, dtype=np.float32)[None, :]
    j = np.floor(cols / 2)
    bq_ = cols % 2
    ebias = (M_SLOPE * (128.0 * j + i) - 64.0 * bq_).astype(np.float32)
    umask = (np.arange(128)[None, :] >= np.arange(128)[:, None]).astype(np.float32)
    return ebias, umask


def kernel(x, Wq, bq, Wk, bk, Wv, bv, Wo, bo, trace=False, _timings=None):
    x = np.ascontiguousarray(np.asarray(x, dtype=np.float32))
    Wq = np.asarray(Wq, dtype=np.float32)
    Wk = np.asarray(Wk, dtype=np.float32)
    Wv = np.asarray(Wv, dtype=np.float32)
    Wo = np.asarray(Wo, dtype=np.float32)
    bq = np.asarray(bq, dtype=np.float32)
    bk = np.asarray(bk, dtype=np.float32)
    bv = np.asarray(bv, dtype=np.float32)
    bo = np.asarray(bo, dtype=np.float32)

    nc = _get_nc()
    ebias, umask = _host_tables()
    xT = [np.ascontiguousarray(x[b].T) for b in range(B)]
    in_maps = []
    for c in range(8):
        b, g = divmod(c, 4)
        fs = slice(FPC * g, FPC * (g + 1))
        in_maps.append(
            {
                "xT": xT[b],
                "wq": np.ascontiguousarray((Wq[fs] / 8.0).T),
                "wk": np.ascontiguousarray(Wk[fs].T),
                "wv": np.ascontiguousarray(Wv[fs].T),
                "bqv": np.ascontiguousarray((bq[fs] / 8.0).reshape(2, 128).T),
                "bkv": np.ascontiguousarray(bk[fs].reshape(2, 128).T),
                "bvv": np.ascontiguousarray(bv[fs].reshape(2, 128).T),
                "wo": np.ascontiguousarray(Wo[:, fs].T),
                "ebias": ebias,
                "umask": umask,
            }
        )
    res = run_bass_kernel_spmd(nc, in_maps, core_ids=list(range(8)), trace=trace)
    if _timings is not None:
        _timings["exec_time_ns"] = res.exec_time_ns
        _timings["profile_json"] = res.profile_json
    out = np.zeros((B, T, D), dtype=np.float32)
    for c in range(8):
        out[c // 4] += res.results[c]["out"]
    out += bo[None, None, :]
    return out
